# revision 1
# baseline (speedup 1.0000x reference)
"""Trainium2 Bass kernel for nn_DemandTemporalEncoder.

TCN (6 dilated causal conv blocks) + sparse top-p attention, data-parallel
over batch across 8 NeuronCores (1 batch sample per core).

Key algebraic facts used:
  * Only attn_out[:, -1, :] is consumed, so attention needs just one query
    (the last position): a single score row s[t] = q . k_t / sqrt(D).
  * s = (Wk^T q) . z_t + q.bk ; the constant q.bk shifts every score equally
    and cancels in both top-k selection and softmax, so K is never built.
  * top-512-of-2048 is computed exactly via rank counting:
    rank_i = #{j : s_j > s_i}; keep rank < 512 (ties have measure zero).
  * exp without max-subtraction: scores are O(0.1), and softmax is
    shift-invariant so the result matches the reference's stabilized form.

Layouts (per core):
  * Activations channel-major in SBUF: [128 part = channel-in-chunk,
    4 chunks x (64 pad + 2048 t)] so a dilated causal shift is a column
    offset and the zero left-pad implements causal padding.
  * Conv = sum of 3 shifted matmuls, contraction over channel chunks on
    the PE partition axis, N=512 time-tile per PSUM bank. Conv weights and
    activations are bf16 (fp32 PSUM accumulation); the score row and the
    softmax weights stay 32-bit. Host-side weight prep also folds
    Wu = Wk^T Wq and Wpv = Wp Wv so the attention tail is two matvecs, one
    score row, a rank scan (split across DVE and ACT), and one w @ V' pass.
"""

import sys

if '/opt/trn_rl_repo' not in sys.path:
    sys.path.insert(0, '/opt/trn_rl_repo')

import numpy as np

B, T, D_IN, D, KS = 8, 2048, 64, 512, 3
N_LAYERS = 6
PAD = 64            # max dilation (32) * (KS-1)
CT = PAD + T        # padded time extent per channel chunk
NCH = 4             # 512 / 128 channel chunks
NTT = 4             # time tiles of 512 for matmul free dim
NTC = 16            # time chunks of 128 for attention
K_KEEP = 512        # int(0.25 * T)
SQRT_D_INV = 1.0 / float(np.sqrt(np.float32(D)))

_CACHE = {}


def _build_program(debug_taps=False):
    import concourse.tile as tile
    from concourse import bacc, mybir
    from contextlib import ExitStack

    F32 = mybir.dt.float32
    F32R = mybir.dt.float32r
    BF16 = mybir.dt.bfloat16
    AF = mybir.ActivationFunctionType
    ALU = mybir.AluOpType

    nc = bacc.Bacc("TRN2", target_bir_lowering=False, debug=False, num_devices=8)

    xcm_d = nc.dram_tensor("xcm", [128, CT], BF16, kind="ExternalInput")
    w0c1_d = nc.dram_tensor("w0c1", [128, 2 * D], BF16, kind="ExternalInput")
    wres_d = nc.dram_tensor("wres", [D_IN, D], BF16, kind="ExternalInput")
    wmain_d = nc.dram_tensor("wmain", [11, 128, NCH * KS * D], BF16, kind="ExternalInput")
    wu_d = nc.dram_tensor("wu", [128, NCH * D], BF16, kind="ExternalInput")
    wpv_d = nc.dram_tensor("wpv", [128, NCH * D], BF16, kind="ExternalInput")
    wp_d = nc.dram_tensor("wp", [128, NCH * D], F32, kind="ExternalInput")
    bcol_d = nc.dram_tensor("bcol", [128, 17 * NCH], F32, kind="ExternalInput")
    ones_d = nc.dram_tensor("ones", [1, 128], F32R, kind="ExternalInput")
    zpad_d = nc.dram_tensor("zpad", [128, PAD], BF16, kind="ExternalInput")
    out_d = nc.dram_tensor("out", [D], F32, kind="ExternalOutput")
    dbg = {}
    if debug_taps:
        for nm, shp, dt_ in [("dbg_y1b0", [128, NCH * CT], BF16), ("dbg_h0", [128, NCH * CT], BF16),
                        ("dbg_z", [128, NCH * CT], BF16), ("dbg_srow", [1, T], F32),
                        ("dbg_spt", [128, NTC], F32), ("dbg_rank", [128, NTC], F32),
                        ("dbg_wpt", [128, NTC], BF16), ("dbg_qcol", [128, NCH], F32),
                        ("dbg_ucol", [128, NCH], BF16), ("dbg_vtm", [128, NTC * D], BF16),
                        ("dbg_orow", [1, D], F32), ("dbg_zsum", [128, NCH], F32),
                        ("dbg_sbcast", [128, T], F32)]:
            dbg[nm] = nc.dram_tensor(nm, shp, dt_, kind="ExternalOutput")

    def r(ap):
        return ap

    def f32(ap):
        return ap.bitcast(F32)

    with ExitStack() as ctx:
        tc = ctx.enter_context(tile.TileContext(nc))
        const = ctx.enter_context(tc.tile_pool(name="const", bufs=1))
        wpool = ctx.enter_context(tc.tile_pool(name="w", bufs=2))
        hpool = ctx.enter_context(tc.tile_pool(name="h", bufs=1))
        ypool = ctx.enter_context(tc.tile_pool(name="y", bufs=1))
        epool = ctx.enter_context(tc.tile_pool(name="e", bufs=4))
        spool = ctx.enter_context(tc.tile_pool(name="s", bufs=1))
        psacc = ctx.enter_context(tc.tile_pool(name="psacc", bufs=4, space="PSUM"))
        psaux = ctx.enter_context(tc.tile_pool(name="psaux", bufs=4, space="PSUM"))
        dpool = ctx.enter_context(tc.tile_pool(name="dram", bufs=1, space="DRAM"))

        xsb = const.tile([128, CT], BF16, tag="x")
        nc.sync.dma_start(xsb[:, 0:PAD + 1024], xcm_d.ap()[:, 0:PAD + 1024])
        nc.sync.dma_start(xsb[:, PAD + 1024:CT], xcm_d.ap()[:, PAD + 1024:CT])
        w0c1 = const.tile([128, 2 * D], BF16, tag="w0c1")
        nc.sync.dma_start(w0c1[:], w0c1_d.ap()[:])
        bcol = const.tile([128, 17 * NCH], F32, tag="bcol")
        nc.scalar.dma_start(bcol[:], bcol_d.ap()[:])
        wres = const.tile([D_IN, D], BF16, tag="wres")
        nc.scalar.dma_start(wres[:], wres_d.ap()[:])
        ones1 = const.tile([1, 128], F32R, tag="ones1")
        nc.scalar.dma_start(ones1[:], ones_d.ap()[:])
        ones128 = const.tile([128, 1], F32, tag="ones128")
        nc.vector.memset(ones128[:], 1.0)

        h = hpool.tile([128, NCH * CT], BF16, tag="h")
        y1 = ypool.tile([128, NCH * CT], BF16, tag="y")
        for cc in range(NCH):
            nc.scalar.dma_start(h[:, cc * CT:cc * CT + PAD], zpad_d.ap()[:])
            nc.scalar.dma_start(y1[:, cc * CT:cc * CT + PAD], zpad_d.ap()[:])

        def bias_ap(vi, mo):
            return bcol[:, vi * NCH + mo:vi * NCH + mo + 1]

        # ------- block 0 conv1: taps (t, t-1) packed on 128 partitions -------
        # xsb: part 0-63 = x(t), part 64-127 = x(t-1); tap t-2 reads part
        # 0-63 at column offset -2. Two matmuls per 512-col psum group.
        for tt in range(NTT):
            for mo in range(NCH):
                pt = psacc.tile([128, 512], F32, tag="acc", name=f"c1_{tt}_{mo}")
                a = PAD + tt * 512
                nc.tensor.matmul(pt[:], w0c1[:, mo * 128:mo * 128 + 128],
                                 xsb[:, a:a + 512], start=True, stop=False)
                nc.tensor.matmul(pt[:], w0c1[0:D_IN, D + mo * 128:D + mo * 128 + 128],
                                 xsb[0:D_IN, a - 2:a + 510], start=False, stop=True)
                nc.scalar.activation(y1[:, mo * CT + a:mo * CT + a + 512],
                                     pt[:], AF.Gelu, bias=bias_ap(0, mo))

        if debug_taps:
            nc.sync.dma_start(dbg["dbg_y1b0"].ap()[:], y1[:])

        # ------------- block 0 conv2 + 1x1 residual -> h, dil=1 -------------
        wsb = wpool.tile([128, NCH * KS * D], BF16, tag="w")
        nc.sync.dma_start(wsb[:], wmain_d.ap()[0])
        # attention weight packs: needed ~500us in; stream them during convs
        packA = const.tile([128, NCH * D], BF16, tag="packa")
        nc.sync.dma_start(packA[:], wu_d.ap()[:])
        packV = const.tile([128, NCH * D], BF16, tag="packv")
        nc.sync.dma_start(packV[:], wpv_d.ap()[:])
        packP = const.tile([128, NCH * D], F32, tag="packp")
        nc.sync.dma_start(packP[:], wp_d.ap()[:])
        for mo in range(NCH):
            pts = [psacc.tile([128, 512], F32, tag="acc", name=f"acc{mo}_{_t}") for _t in range(NTT)]
            idx = 0
            for cc in range(NCH):
                for k in range(KS):
                    lhsT = r(wsb[:, (cc * KS + k) * D + mo * 128:(cc * KS + k) * D + mo * 128 + 128])
                    for tt in range(NTT):
                        a = cc * CT + PAD + tt * 512 - k
                        nc.tensor.matmul(pts[tt][:], lhsT, r(y1[:, a:a + 512]),
                                         start=(idx == 0), stop=(idx == NCH * KS - 1))
                    idx += 1
            for tt in range(NTT):
                y2t = epool.tile([128, 512], F32, tag="y2")
                nc.scalar.activation(y2t[:], pts[tt][:], AF.Gelu, bias=bias_ap(1, mo))
                pr = psaux.tile([128, 512], F32, tag="aux")
                nc.tensor.matmul(pr[:], r(wres[:, mo * 128:mo * 128 + 128]),
                                 r(xsb[0:D_IN, PAD + tt * 512:PAD + tt * 512 + 512]),
                                 start=True, stop=True)
                rt = epool.tile([128, 512], F32, tag="res")
                nc.scalar.activation(rt[:], pr[:], AF.Identity, bias=bias_ap(2, mo))
                dst = h[:, mo * CT + PAD + tt * 512:mo * CT + PAD + tt * 512 + 512]
                nc.vector.tensor_add(dst, y2t[:], rt[:])

        if debug_taps:
            nc.sync.dma_start(dbg["dbg_h0"].ap()[:], h[:])

        # ---------------- blocks 1..5: identity residual ----------------
        for i in range(N_LAYERS - 1):
            dil = 2 ** (i + 1)
            for half in range(2):
                widx = 1 + 2 * i + half
                vi = 3 + 2 * i + half
                src = h if half == 0 else y1
                wsb = wpool.tile([128, NCH * KS * D], BF16, tag="w")
                nc.sync.dma_start(wsb[:], wmain_d.ap()[widx])
                for mo in range(NCH):
                    pts = [psacc.tile([128, 512], F32, tag="acc", name=f"acc{mo}_{_t}") for _t in range(NTT)]
                    idx = 0
                    for cc in range(NCH):
                        for k in range(KS):
                            lhsT = r(wsb[:, (cc * KS + k) * D + mo * 128:(cc * KS + k) * D + mo * 128 + 128])
                            for tt in range(NTT):
                                a = cc * CT + PAD + tt * 512 - k * dil
                                nc.tensor.matmul(pts[tt][:], lhsT, r(src[:, a:a + 512]),
                                                 start=(idx == 0), stop=(idx == NCH * KS - 1))
                            idx += 1
                    for tt in range(NTT):
                        dsl = slice(mo * CT + PAD + tt * 512, mo * CT + PAD + tt * 512 + 512)
                        if half == 0:
                            nc.scalar.activation(y1[:, dsl], pts[tt][:], AF.Gelu,
                                                 bias=bias_ap(vi, mo))
                        else:
                            y2t = epool.tile([128, 512], F32, tag="y2")
                            nc.scalar.activation(y2t[:], pts[tt][:], AF.Gelu,
                                                 bias=bias_ap(vi, mo))
                            nc.vector.tensor_add(h[:, dsl], h[:, dsl], y2t[:])

        if debug_taps:
            nc.sync.dma_start(dbg["dbg_z"].ap()[:], h[:])

        # ---------------- attention (z = h) ----------------
        zlast = spool.tile([128, NCH], BF16, tag="zlast")
        zl_src = h[:].rearrange("p (c t) -> p c t", c=NCH)[:, :, PAD + T - 1]
        nc.vector.tensor_copy(zlast[:], zl_src)
        # zlb = z_last + bv (bv folds out of the attention sum: weights sum to 1)
        zlb = spool.tile([128, NCH], F32, tag="zlb")
        nc.vector.tensor_add(zlb[:], zlast[:], bcol[:, 15 * NCH:16 * NCH])


        # u = Wu z_last + bu, Wu = Wk^T Wq, bu = Wk^T bq (host-folded).
        # The q.bk constant cancels in softmax/top-k.
        pu = psaux.tile([128, NCH], F32, tag="aux")
        for mc in range(NCH):
            for cc in range(NCH):
                nc.tensor.matmul(pu[:, mc:mc + 1], packA[:, cc * D + mc * 128:cc * D + mc * 128 + 128],
                                 zlast[:, cc:cc + 1], start=(cc == 0), stop=(cc == NCH - 1))
        ucol = spool.tile([128, NCH], BF16, tag="ucol")
        nc.vector.tensor_add(ucol[:], pu[:], bcol[:, 13 * NCH:14 * NCH])

        # s[t] = u . z_t / sqrt(D)   (row layout [1, 2048])
        srow = spool.tile([1, T], F32R, tag="srow")
        for tt in range(NTT):
            psm = psaux.tile([1, 512], F32, tag="aux")
            for cc in range(NCH):
                nc.tensor.matmul(psm[:], r(ucol[:, cc:cc + 1]),
                                 r(h[:, cc * CT + PAD + tt * 512:cc * CT + PAD + tt * 512 + 512]),
                                 start=(cc == 0), stop=(cc == NCH - 1))
            nc.scalar.mul(srow[0:1, tt * 512:tt * 512 + 512], psm[:], SQRT_D_INV)

        # s in [128, 16] layout: (p, ci) = s[ci*128 + p].
        # Bounced through DRAM per time-tile so the transpose pipelines
        # behind the s-stage instead of serializing after it.
        sbounce = dpool.tile([1, T], F32, tag="sbounce")
        spt = spool.tile([128, NTC], F32, tag="spt")
        for tt in range(NTT):
            nc.gpsimd.dma_start(sbounce[0:1, tt * 512:tt * 512 + 512],
                                f32(srow[0:1, tt * 512:tt * 512 + 512]))
            nc.gpsimd.dma_start(
                spt[:, tt * NCH:(tt + 1) * NCH],
                sbounce[0:1, tt * 512:tt * 512 + 512].rearrange("a (b c) -> (a c) b", b=NCH))

        # broadcast s to all partitions via K=1 outer-product matmul
        sbcast = spool.tile([128, T], F32, tag="sbcast")
        for tt in range(NTT):
            pb = psacc.tile([128, 512], F32, tag="acc")
            nc.tensor.matmul(pb[:], r(ones1[:]), r(srow[0:1, tt * 512:tt * 512 + 512]),
                             start=True, stop=True)
            nc.scalar.copy(sbcast[:, tt * 512:tt * 512 + 512], pb[:])

        # V' tiles: V'_tm[ci] = (z_chunk)^T @ (Wp Wv)^T -> [t 128, d 512]
        # (Wpv host-folded; evacuations split DVE/ACT to balance the tail)
        vtm = ypool.tile([128, NTC * D], BF16, tag="y")
        vps = []
        for ci in range(NTC):
            pv = psacc.tile([128, 512], F32, tag="acc", name=f"vps{ci}")
            for cc in range(NCH):
                nc.tensor.matmul(pv[:], r(h[:, cc * CT + PAD + ci * 128:cc * CT + PAD + ci * 128 + 128]),
                                 packV[:, cc * D:cc * D + D],
                                 start=(cc == 0), stop=(cc == NCH - 1))
            vps.append(pv)

        def v_evac(ci, eng):
            eng(vtm[:, ci * D:ci * D + D], vps[ci][:])

        # exp(s) does not depend on the rank scan: compute it up front
        ept = spool.tile([128, NTC], F32, tag="ept")
        nc.scalar.activation(ept[:], spt[:], AF.Exp)
        negspt = spool.tile([128, 8], F32, tag="negspt")
        nc.vector.tensor_scalar_mul(negspt[:], spt[:, 8:NTC], -1.0)

        # pre = Wp @ (z_last + bv) + bp, bounced to row layout off-critical-path
        ppre = psaux.tile([128, NCH], F32, tag="aux")
        for mo in range(NCH):
            for cc in range(NCH):
                nc.tensor.matmul(ppre[:, mo:mo + 1], packP[:, cc * D + mo * 128:cc * D + mo * 128 + 128],
                                 zlb[:, cc:cc + 1], start=(cc == 0), stop=(cc == NCH - 1))
        pre = spool.tile([128, NCH], F32, tag="pre")
        nc.vector.tensor_add(pre[:], ppre[:], bcol[:, 16 * NCH:17 * NCH])
        pbounce = dpool.tile([128, NCH], F32, tag="pbounce")
        nc.gpsimd.dma_start(pbounce[:], pre[:])
        prerow = spool.tile([1, D], F32, tag="prerow")
        nc.gpsimd.dma_start(prerow[:], pbounce[:].rearrange("a b -> b a"))

        # V' evacuations first (both engines), so vtm is ready when po needs it
        for ci in range(NTC):
            if ci % 2 == 0:
                v_evac(ci, lambda o, i: nc.vector.tensor_copy(o, i))
            else:
                v_evac(ci, lambda o, i: nc.scalar.copy(o, i))

        # rank_i = #{j : s_j > s_i}: 8 sweeps on DVE (is_gt) + 8 on ACT (Sign)
        junk = spool.tile([128, T], F32, tag="junk")
        junkA = spool.tile([128, T], F32, tag="junkA")
        rank = spool.tile([128, NTC], F32, tag="rank")
        sgn = spool.tile([128, 8], F32, tag="sgn")
        for ci in range(8):
            nc.vector.tensor_scalar(junk[:], sbcast[:], spt[:, ci:ci + 1], None,
                                    op0=ALU.is_gt, op1=ALU.add,
                                    accum_out=rank[:, ci:ci + 1])
        for ci in range(8):
            nc.scalar.activation(junkA[:], sbcast[:], AF.Sign,
                                 bias=negspt[:, ci:ci + 1],
                                 accum_out=sgn[:, ci:ci + 1])
        # sum(sign(s_j - s_i)) = #greater - #less; with self(=0): rank = (sgn+2047)/2
        nc.vector.tensor_scalar(rank[:, 8:NTC], sgn[:], float(T - 1), 0.5,
                                op0=ALU.add, op1=ALU.mult)

        # w = exp(s) * (rank < 512)
        keep = spool.tile([128, NTC], F32, tag="keep")
        nc.vector.tensor_scalar(keep[:], rank[:], float(K_KEEP) - 0.5, None, op0=ALU.is_lt)
        wpt = spool.tile([128, NTC], BF16, tag="wpt")
        nc.vector.tensor_mul(wpt[:], ept[:], keep[:])

        # Z = sum(w); 1/Z
        wsum = spool.tile([128, 1], F32, tag="wsum")
        nc.vector.reduce_sum(wsum[:], wpt[:], axis=mybir.AxisListType.X)
        pz = psaux.tile([1, 1], F32, tag="aux")
        nc.tensor.matmul(pz[:], wsum[:], ones128[:], start=True, stop=True)
        rz = spool.tile([1, 1], F32, tag="rz")
        nc.vector.reciprocal(rz[:], pz[:])

        # out = pre + (w @ V') / Z   -- all in row layout on partition 0
        po = psaux.tile([1, 512], F32, tag="aux")
        for ci in range(NTC):
            nc.tensor.matmul(po[:], r(wpt[:, ci:ci + 1]), r(vtm[:, ci * D:ci * D + D]),
                             start=(ci == 0), stop=(ci == NTC - 1))
        outrow = spool.tile([1, D], F32, tag="outrow")
        nc.vector.tensor_scalar(outrow[:], po[:], rz[:], None, op0=ALU.mult)
        nc.vector.tensor_add(outrow[:], outrow[:], prerow[:])
        if debug_taps:
            nc.sync.dma_start(dbg["dbg_srow"].ap()[:], f32(srow[:]))
            nc.sync.dma_start(dbg["dbg_spt"].ap()[:], spt[:])
            nc.sync.dma_start(dbg["dbg_wpt"].ap()[:], wpt[:])
            nc.sync.dma_start(dbg["dbg_qcol"].ap()[:], zlast[:])
            nc.sync.dma_start(dbg["dbg_ucol"].ap()[:], ucol[:])
            nc.sync.dma_start(dbg["dbg_vtm"].ap()[:], vtm[:])
            nc.sync.dma_start(dbg["dbg_zsum"].ap()[:], pre[:])
        nc.sync.dma_start(out_d.ap()[None, :], outrow[:])

    nc.compile()
    return nc


def get_program(debug_taps=False):
    key = 'nc_dbg' if debug_taps else 'nc'
    if key not in _CACHE:
        _CACHE[key] = _build_program(debug_taps)
    return _CACHE[key]


def _pack_chunked(w):
    """[d_out, c_in] (512x512) -> [128, 4*512] with [p, cc*512+m] = w[cc*128+p, m].

    Pass w already oriented so that rows are the matmul contraction dim.
    """
    return np.ascontiguousarray(
        w.reshape(NCH, 128, D).transpose(1, 0, 2).reshape(128, NCH * D))


def _pack_conv(w):
    """[C_out, C_in=512, KS] -> [128, (cc, k, m)] with
    [p, (cc*KS+k)*512 + m] = w[m, cc*128+p, KS-1-k].

    Taps are stored reversed: XLA conv (cross-correlation) applies tap j to
    x[t - (KS-1-j)*dil], while the kernel shifts tap k by k*dil.
    """
    wt = w[:, :, ::-1].transpose(1, 2, 0)           # [cin, k, cout]
    wt = wt.reshape(NCH, 128, KS, D).transpose(1, 0, 2, 3)  # [p, cc, k, m]
    return np.ascontiguousarray(wt.reshape(128, NCH * KS * D))


def _bias_col(v):
    return np.ascontiguousarray(v.reshape(NCH, 128).T)


def make_in_maps(x, c1w0, c1b0, c2w0, c2b0, resw, resb, c1w, c1b, c2w, c2b,
                 wq, bq, wk, bk, wv, bv, wp, bp):
    import ml_dtypes
    bf16 = ml_dtypes.bfloat16
    f = lambda a: np.asarray(a, dtype=np.float32)
    x = f(x)

    # block0 conv1 taps packed: part 0-63 tap@t (w[..,2]), part 64-127
    # tap@t-1 (w[..,1]); second D block, part 0-63: tap@t-2 (w[..,0]).
    w0 = f(c1w0)
    w0c1 = np.zeros((128, 2 * D), np.float32)
    w0c1[0:64, 0:D] = w0[:, :, 2].T
    w0c1[64:128, 0:D] = w0[:, :, 1].T
    w0c1[0:64, D:2 * D] = w0[:, :, 0].T
    w0c1 = w0c1.astype(bf16)
    wres_p = np.ascontiguousarray(f(resw)[:, :, 0].T).astype(bf16)

    convs = [f(c2w0)]
    for i in range(N_LAYERS - 1):
        convs.append(f(c1w)[i])
        convs.append(f(c2w)[i])
    wmain = np.stack([_pack_conv(w) for w in convs]).astype(bf16)

    wu = (f(wk).astype(np.float64).T @ f(wq).astype(np.float64)).astype(np.float32)
    wpv = (f(wp).astype(np.float64) @ f(wv).astype(np.float64)).astype(np.float32)
    wu_p = _pack_chunked(wu.T).astype(bf16)         # lhsT tiles for u = Wu z_last
    wpv_p = _pack_chunked(wpv.T).astype(bf16)       # rhs tiles for V' = (Wp Wv) z
    wp_p = _pack_chunked(f(wp).T)                   # lhsT tiles for pre

    bvecs = [f(c1b0), f(c2b0), f(resb)]
    for i in range(N_LAYERS - 1):
        bvecs.append(f(c1b)[i])
        bvecs.append(f(c2b)[i])
    bu = (f(wk).astype(np.float64).T @ f(bq).astype(np.float64)).astype(np.float32)
    bvecs += [bu, f(bk), f(bv), f(bp)]
    bcol = np.concatenate([_bias_col(v) for v in bvecs], axis=1)

    in_maps = []
    for b in range(B):
        xb = x[b].T  # [64, T]
        xcm = np.zeros((128, CT), np.float32)
        xcm[0:64, PAD:] = xb
        xcm[64:128, PAD:] = np.pad(xb, ((0, 0), (1, 0)))[:, :T]   # x(t-1)
        in_maps.append({
            "xcm": np.ascontiguousarray(xcm).astype(bf16),
            "ones": np.ones((1, 128), np.float32),
            "zpad": np.zeros((128, PAD), bf16),
            "w0c1": w0c1,
            "wres": wres_p,
            "wmain": wmain,
            "wu": wu_p,
            "wpv": wpv_p,
            "wp": wp_p,
            "bcol": bcol,
        })
    return in_maps


def _enable_ldw_opt():
    """Let walrus dedupe consecutive same-stationary LDWEIGHTS: our conv
    emits 4 back-to-back matmuls per weight tile, so the reload is pure
    overhead. The flag is hardcoded off in bass_utils; rewrite it on the
    walrus_driver command line for our own compiles."""
    from concourse import bass_utils
    if getattr(bass_utils, '_ldw_patched', False):
        return
    orig = bass_utils.run_command

    def run_command_ldw(cmd, cwd=None, **kw):
        cmd = ['--enable-ldw-opt=true' if c == '--enable-ldw-opt=false' else c
               for c in cmd]
        return orig(cmd, cwd=cwd, **kw)

    bass_utils.run_command = run_command_ldw
    bass_utils._ldw_patched = True


def kernel(**inputs):
    from concourse import bass_utils
    nc = get_program()
    in_maps = make_in_maps(**inputs)
    res = bass_utils.run_bass_kernel_spmd(nc, in_maps, core_ids=list(range(B)))
    out = np.stack([res.results[b]["out"] for b in range(B)], axis=0)
    return out.astype(np.float32)



# revision 3
# speedup vs baseline: 1.2021x; 1.2021x over previous
"""Trainium2 Bass kernel for nn_DemandTemporalEncoder.

TCN (6 dilated causal conv blocks) + sparse top-p attention, data-parallel
over batch across 8 NeuronCores (1 batch sample per core).

Key algebraic facts used:
  * Only attn_out[:, -1, :] is consumed, so attention needs just one query
    (the last position): a single score row s[t] = q . k_t / sqrt(D).
  * s = (Wk^T q) . z_t + q.bk ; the constant q.bk shifts every score equally
    and cancels in both top-k selection and softmax, so K is never built.
  * top-512-of-2048 is computed exactly via rank counting:
    rank_i = #{j : s_j > s_i}; keep rank < 512 (ties have measure zero).
  * exp without max-subtraction: scores are O(0.1), and softmax is
    shift-invariant so the result matches the reference's stabilized form.

Layouts (per core):
  * Activations channel-major in SBUF: [128 part = channel-in-chunk,
    4 chunks x (64 pad + 2048 t)] so a dilated causal shift is a column
    offset and the zero left-pad implements causal padding.
  * Conv = sum of 3 shifted matmuls, contraction over channel chunks on
    the PE partition axis, N=512 time-tile per PSUM bank. Conv weights and
    activations are bf16 (fp32 PSUM accumulation); the score row and the
    softmax weights stay 32-bit. Host-side weight prep also folds
    Wu = Wk^T Wq and Wpv = Wp Wv so the attention tail is two matvecs, one
    score row, a rank scan (split across DVE and ACT), and one w @ V' pass.
"""

import sys

if '/opt/trn_rl_repo' not in sys.path:
    sys.path.insert(0, '/opt/trn_rl_repo')

import numpy as np

B, T, D_IN, D, KS = 8, 2048, 64, 512, 3
N_LAYERS = 6
PAD = 64            # max dilation (32) * (KS-1)
CT = PAD + T        # padded time extent per channel chunk
NCH = 4             # 512 / 128 channel chunks
NTT = 4             # time tiles of 512 for matmul free dim
NTC = 16            # time chunks of 128 for attention
K_KEEP = 512        # int(0.25 * T)
SQRT_D_INV = 1.0 / float(np.sqrt(np.float32(D)))

_CACHE = {}


def _build_program(debug_taps=False):
    import concourse.tile as tile
    from concourse import bacc, mybir
    from contextlib import ExitStack

    F32 = mybir.dt.float32
    F32R = mybir.dt.float32r
    BF16 = mybir.dt.bfloat16
    AF = mybir.ActivationFunctionType
    ALU = mybir.AluOpType

    nc = bacc.Bacc("TRN2", target_bir_lowering=False, debug=False, num_devices=8)

    xcm_d = nc.dram_tensor("xcm", [128, CT], BF16, kind="ExternalInput")
    w0c1_d = nc.dram_tensor("w0c1", [128, 2 * D], BF16, kind="ExternalInput")
    wres_d = nc.dram_tensor("wres", [D_IN, D], BF16, kind="ExternalInput")
    wmain_d = nc.dram_tensor("wmain", [11, 128, NCH * KS * D], BF16, kind="ExternalInput")
    wu_d = nc.dram_tensor("wu", [128, NCH * D], BF16, kind="ExternalInput")
    wpv_d = nc.dram_tensor("wpv", [128, NCH * D], BF16, kind="ExternalInput")
    wp_d = nc.dram_tensor("wp", [128, NCH * D], F32, kind="ExternalInput")
    bcol_d = nc.dram_tensor("bcol", [128, 17 * NCH], F32, kind="ExternalInput")
    ones_d = nc.dram_tensor("ones", [1, 128], F32R, kind="ExternalInput")
    zpad_d = nc.dram_tensor("zpad", [128, PAD], BF16, kind="ExternalInput")
    out_d = nc.dram_tensor("out", [D], F32, kind="ExternalOutput")
    dbg = {}
    if debug_taps:
        for nm, shp, dt_ in [("dbg_y1b0", [128, NCH * CT], BF16), ("dbg_h0", [128, NCH * CT], BF16),
                        ("dbg_z", [128, NCH * CT], BF16), ("dbg_srow", [1, T], F32),
                        ("dbg_spt", [128, NTC], F32), ("dbg_rank", [128, NTC], F32),
                        ("dbg_wpt", [128, NTC], BF16), ("dbg_qcol", [128, NCH], F32),
                        ("dbg_ucol", [128, NCH], BF16), ("dbg_vtm", [128, NTC * D], BF16),
                        ("dbg_orow", [1, D], F32), ("dbg_zsum", [128, NCH], F32),
                        ("dbg_sbcast", [128, T], F32)]:
            dbg[nm] = nc.dram_tensor(nm, shp, dt_, kind="ExternalOutput")

    def r(ap):
        return ap

    def f32(ap):
        return ap.bitcast(F32)

    with ExitStack() as ctx:
        tc = ctx.enter_context(tile.TileContext(nc))
        const = ctx.enter_context(tc.tile_pool(name="const", bufs=1))
        wpool = ctx.enter_context(tc.tile_pool(name="w", bufs=2))
        hpool = ctx.enter_context(tc.tile_pool(name="h", bufs=1))
        ypool = ctx.enter_context(tc.tile_pool(name="y", bufs=1))
        epool = ctx.enter_context(tc.tile_pool(name="e", bufs=4))
        spool = ctx.enter_context(tc.tile_pool(name="s", bufs=1))
        psacc = ctx.enter_context(tc.tile_pool(name="psacc", bufs=4, space="PSUM"))
        psaux = ctx.enter_context(tc.tile_pool(name="psaux", bufs=4, space="PSUM"))
        dpool = ctx.enter_context(tc.tile_pool(name="dram", bufs=1, space="DRAM"))

        xsb = const.tile([128, CT], BF16, tag="x")
        nc.sync.dma_start(xsb[:, 0:PAD + 1024], xcm_d.ap()[:, 0:PAD + 1024])
        nc.sync.dma_start(xsb[:, PAD + 1024:CT], xcm_d.ap()[:, PAD + 1024:CT])
        w0c1 = const.tile([128, 2 * D], BF16, tag="w0c1")
        nc.sync.dma_start(w0c1[:], w0c1_d.ap()[:])
        bcol = const.tile([128, 17 * NCH], F32, tag="bcol")
        nc.scalar.dma_start(bcol[:], bcol_d.ap()[:])
        wres = const.tile([D_IN, D], BF16, tag="wres")
        nc.scalar.dma_start(wres[:], wres_d.ap()[:])
        ones1 = const.tile([1, 128], F32R, tag="ones1")
        nc.scalar.dma_start(ones1[:], ones_d.ap()[:])
        ones128 = const.tile([128, 1], F32, tag="ones128")
        nc.vector.memset(ones128[:], 1.0)

        h = hpool.tile([128, NCH * CT], BF16, tag="h")
        y1 = ypool.tile([128, NCH * CT], BF16, tag="y")
        for cc in range(NCH):
            nc.scalar.dma_start(h[:, cc * CT:cc * CT + PAD], zpad_d.ap()[:])
            nc.scalar.dma_start(y1[:, cc * CT:cc * CT + PAD], zpad_d.ap()[:])

        def bias_ap(vi, mo):
            return bcol[:, vi * NCH + mo:vi * NCH + mo + 1]

        # ------- block 0 conv1: taps (t, t-1) packed on 128 partitions -------
        # xsb: part 0-63 = x(t), part 64-127 = x(t-1); tap t-2 reads part
        # 0-63 at column offset -2. Two matmuls per 512-col psum group.
        for tt in range(NTT):
            for mo in range(NCH):
                pt = psacc.tile([128, 512], F32, tag="acc", name=f"c1_{tt}_{mo}")
                a = PAD + tt * 512
                nc.tensor.matmul(pt[:], w0c1[:, mo * 128:mo * 128 + 128],
                                 xsb[:, a:a + 512], start=True, stop=False)
                nc.tensor.matmul(pt[:], w0c1[0:D_IN, D + mo * 128:D + mo * 128 + 128],
                                 xsb[0:D_IN, a - 2:a + 510], start=False, stop=True)
                nc.scalar.activation(y1[:, mo * CT + a:mo * CT + a + 512],
                                     pt[:], AF.Gelu, bias=bias_ap(0, mo))

        if debug_taps:
            nc.sync.dma_start(dbg["dbg_y1b0"].ap()[:], y1[:])

        # ------------- block 0 conv2 + 1x1 residual -> h, dil=1 -------------
        wsb = wpool.tile([128, NCH * KS * D], BF16, tag="w")
        nc.sync.dma_start(wsb[:], wmain_d.ap()[0])
        # attention weight packs: needed ~500us in; stream them during convs
        packA = const.tile([128, NCH * D], BF16, tag="packa")
        nc.sync.dma_start(packA[:], wu_d.ap()[:])
        packV = const.tile([128, NCH * D], BF16, tag="packv")
        nc.sync.dma_start(packV[:], wpv_d.ap()[:])
        packP = const.tile([128, NCH * D], F32, tag="packp")
        nc.sync.dma_start(packP[:], wp_d.ap()[:])
        for mo in range(NCH):
            pts = [psacc.tile([128, 512], F32, tag="acc", name=f"acc{mo}_{_t}") for _t in range(NTT)]
            idx = 0
            for cc in range(NCH):
                for k in range(KS):
                    lhsT = r(wsb[:, (cc * KS + k) * D + mo * 128:(cc * KS + k) * D + mo * 128 + 128])
                    for tt in range(NTT):
                        a = cc * CT + PAD + tt * 512 - k
                        nc.tensor.matmul(pts[tt][:], lhsT, r(y1[:, a:a + 512]),
                                         start=(idx == 0), stop=(idx == NCH * KS - 1))
                    idx += 1
            for tt in range(NTT):
                y2t = epool.tile([128, 512], F32, tag="y2")
                nc.scalar.activation(y2t[:], pts[tt][:], AF.Gelu, bias=bias_ap(1, mo))
                pr = psaux.tile([128, 512], F32, tag="aux")
                nc.tensor.matmul(pr[:], r(wres[:, mo * 128:mo * 128 + 128]),
                                 r(xsb[0:D_IN, PAD + tt * 512:PAD + tt * 512 + 512]),
                                 start=True, stop=True)
                rt = epool.tile([128, 512], F32, tag="res")
                nc.scalar.activation(rt[:], pr[:], AF.Identity, bias=bias_ap(2, mo))
                dst = h[:, mo * CT + PAD + tt * 512:mo * CT + PAD + tt * 512 + 512]
                nc.vector.tensor_add(dst, y2t[:], rt[:])

        if debug_taps:
            nc.sync.dma_start(dbg["dbg_h0"].ap()[:], h[:])

        # ---------------- blocks 1..5: identity residual ----------------
        for i in range(N_LAYERS - 1):
            dil = 2 ** (i + 1)
            for half in range(2):
                widx = 1 + 2 * i + half
                vi = 3 + 2 * i + half
                src = h if half == 0 else y1
                wsb = wpool.tile([128, NCH * KS * D], BF16, tag="w")
                nc.sync.dma_start(wsb[:], wmain_d.ap()[widx])
                for mo in range(NCH):
                    pts = [psacc.tile([128, 512], F32, tag="acc", name=f"acc{mo}_{_t}") for _t in range(NTT)]
                    idx = 0
                    for cc in range(NCH):
                        for k in range(KS):
                            lhsT = r(wsb[:, (cc * KS + k) * D + mo * 128:(cc * KS + k) * D + mo * 128 + 128])
                            for tt in range(NTT):
                                a = cc * CT + PAD + tt * 512 - k * dil
                                nc.tensor.matmul(pts[tt][:], lhsT, r(src[:, a:a + 512]),
                                                 start=(idx == 0), stop=(idx == NCH * KS - 1))
                            idx += 1
                    for tt in range(NTT):
                        dsl = slice(mo * CT + PAD + tt * 512, mo * CT + PAD + tt * 512 + 512)
                        if half == 0:
                            nc.scalar.activation(y1[:, dsl], pts[tt][:], AF.Gelu,
                                                 bias=bias_ap(vi, mo))
                        else:
                            y2t = epool.tile([128, 512], F32, tag="y2")
                            nc.scalar.activation(y2t[:], pts[tt][:], AF.Gelu,
                                                 bias=bias_ap(vi, mo))
                            nc.vector.tensor_add(h[:, dsl], h[:, dsl], y2t[:])

        if debug_taps:
            nc.sync.dma_start(dbg["dbg_z"].ap()[:], h[:])

        # ---------------- attention (z = h) ----------------
        zlast = spool.tile([128, NCH], BF16, tag="zlast")
        zl_src = h[:].rearrange("p (c t) -> p c t", c=NCH)[:, :, PAD + T - 1]
        nc.vector.tensor_copy(zlast[:], zl_src)
        # zlb = z_last + bv (bv folds out of the attention sum: weights sum to 1)
        zlb = spool.tile([128, NCH], F32, tag="zlb")
        nc.vector.tensor_add(zlb[:], zlast[:], bcol[:, 15 * NCH:16 * NCH])


        # u = Wu z_last + bu, Wu = Wk^T Wq, bu = Wk^T bq (host-folded).
        # The q.bk constant cancels in softmax/top-k.
        pu = psaux.tile([128, NCH], F32, tag="aux")
        for mc in range(NCH):
            for cc in range(NCH):
                nc.tensor.matmul(pu[:, mc:mc + 1], packA[:, cc * D + mc * 128:cc * D + mc * 128 + 128],
                                 zlast[:, cc:cc + 1], start=(cc == 0), stop=(cc == NCH - 1))
        ucol = spool.tile([128, NCH], BF16, tag="ucol")
        nc.vector.tensor_add(ucol[:], pu[:], bcol[:, 13 * NCH:14 * NCH])

        # s[t] = u . z_t / sqrt(D)   (row layout [1, 2048])
        srow = spool.tile([1, T], F32R, tag="srow")
        for tt in range(NTT):
            psm = psaux.tile([1, 512], F32, tag="aux")
            for cc in range(NCH):
                nc.tensor.matmul(psm[:], r(ucol[:, cc:cc + 1]),
                                 r(h[:, cc * CT + PAD + tt * 512:cc * CT + PAD + tt * 512 + 512]),
                                 start=(cc == 0), stop=(cc == NCH - 1))
            nc.scalar.mul(srow[0:1, tt * 512:tt * 512 + 512], psm[:], SQRT_D_INV)

        # s in [128, 16] layout: (p, ci) = s[ci*128 + p].
        # Bounced through DRAM per time-tile so the transpose pipelines
        # behind the s-stage instead of serializing after it.
        sbounce = dpool.tile([1, T], F32, tag="sbounce")
        spt = spool.tile([128, NTC], F32, tag="spt")
        for tt in range(NTT):
            nc.gpsimd.dma_start(sbounce[0:1, tt * 512:tt * 512 + 512],
                                f32(srow[0:1, tt * 512:tt * 512 + 512]))
            nc.gpsimd.dma_start(
                spt[:, tt * NCH:(tt + 1) * NCH],
                sbounce[0:1, tt * 512:tt * 512 + 512].rearrange("a (b c) -> (a c) b", b=NCH))

        # broadcast s to all partitions via K=1 outer-product matmul
        sbcast = spool.tile([128, T], F32, tag="sbcast")
        for tt in range(NTT):
            pb = psacc.tile([128, 512], F32, tag="acc")
            nc.tensor.matmul(pb[:], r(ones1[:]), r(srow[0:1, tt * 512:tt * 512 + 512]),
                             start=True, stop=True)
            nc.scalar.copy(sbcast[:, tt * 512:tt * 512 + 512], pb[:])

        # V' tiles: V'_tm[ci] = (z_chunk)^T @ (Wp Wv)^T -> [t 128, d 512]
        # (Wpv host-folded; evacuations split DVE/ACT to balance the tail)
        vtm = ypool.tile([128, NTC * D], BF16, tag="y")
        vps = []
        for ci in range(NTC):
            pv = psacc.tile([128, 512], F32, tag="acc", name=f"vps{ci}")
            for cc in range(NCH):
                nc.tensor.matmul(pv[:], r(h[:, cc * CT + PAD + ci * 128:cc * CT + PAD + ci * 128 + 128]),
                                 packV[:, cc * D:cc * D + D],
                                 start=(cc == 0), stop=(cc == NCH - 1))
            vps.append(pv)

        def v_evac(ci, eng):
            eng(vtm[:, ci * D:ci * D + D], vps[ci][:])

        # exp(s) does not depend on the rank scan: compute it up front
        ept = spool.tile([128, NTC], F32, tag="ept")
        nc.scalar.activation(ept[:], spt[:], AF.Exp)
        negspt = spool.tile([128, 8], F32, tag="negspt")
        nc.vector.tensor_scalar_mul(negspt[:], spt[:, 8:NTC], -1.0)

        # pre = Wp @ (z_last + bv) + bp, bounced to row layout off-critical-path
        ppre = psaux.tile([128, NCH], F32, tag="aux")
        for mo in range(NCH):
            for cc in range(NCH):
                nc.tensor.matmul(ppre[:, mo:mo + 1], packP[:, cc * D + mo * 128:cc * D + mo * 128 + 128],
                                 zlb[:, cc:cc + 1], start=(cc == 0), stop=(cc == NCH - 1))
        pre = spool.tile([128, NCH], F32, tag="pre")
        nc.vector.tensor_add(pre[:], ppre[:], bcol[:, 16 * NCH:17 * NCH])
        pbounce = dpool.tile([128, NCH], F32, tag="pbounce")
        nc.gpsimd.dma_start(pbounce[:], pre[:])
        prerow = spool.tile([1, D], F32, tag="prerow")
        nc.gpsimd.dma_start(prerow[:], pbounce[:].rearrange("a b -> b a"))

        # V' evacuations first (both engines), so vtm is ready when po needs it
        for ci in range(NTC):
            if ci % 2 == 0:
                v_evac(ci, lambda o, i: nc.vector.tensor_copy(o, i))
            else:
                v_evac(ci, lambda o, i: nc.scalar.copy(o, i))

        # rank_i = #{j : s_j > s_i}: 8 sweeps on DVE (is_gt) + 8 on ACT (Sign)
        junk = spool.tile([128, T], F32, tag="junk")
        junkA = spool.tile([128, T], F32, tag="junkA")
        rank = spool.tile([128, NTC], F32, tag="rank")
        sgn = spool.tile([128, 8], F32, tag="sgn")
        for ci in range(8):
            nc.vector.tensor_scalar(junk[:], sbcast[:], spt[:, ci:ci + 1], None,
                                    op0=ALU.is_gt, op1=ALU.add,
                                    accum_out=rank[:, ci:ci + 1])
        for ci in range(8):
            nc.scalar.activation(junkA[:], sbcast[:], AF.Sign,
                                 bias=negspt[:, ci:ci + 1],
                                 accum_out=sgn[:, ci:ci + 1])
        # sum(sign(s_j - s_i)) = #greater - #less; with self(=0): rank = (sgn+2047)/2
        nc.vector.tensor_scalar(rank[:, 8:NTC], sgn[:], float(T - 1), 0.5,
                                op0=ALU.add, op1=ALU.mult)

        # w = exp(s) * (rank < 512)
        keep = spool.tile([128, NTC], F32, tag="keep")
        nc.vector.tensor_scalar(keep[:], rank[:], float(K_KEEP) - 0.5, None, op0=ALU.is_lt)
        wpt = spool.tile([128, NTC], BF16, tag="wpt")
        nc.vector.tensor_mul(wpt[:], ept[:], keep[:])

        # Z = sum(w); 1/Z
        wsum = spool.tile([128, 1], F32, tag="wsum")
        nc.vector.reduce_sum(wsum[:], wpt[:], axis=mybir.AxisListType.X)
        pz = psaux.tile([1, 1], F32, tag="aux")
        nc.tensor.matmul(pz[:], wsum[:], ones128[:], start=True, stop=True)
        rz = spool.tile([1, 1], F32, tag="rz")
        nc.vector.reciprocal(rz[:], pz[:])

        # out = pre + (w @ V') / Z   -- all in row layout on partition 0
        po = psaux.tile([1, 512], F32, tag="aux")
        for ci in range(NTC):
            nc.tensor.matmul(po[:], r(wpt[:, ci:ci + 1]), r(vtm[:, ci * D:ci * D + D]),
                             start=(ci == 0), stop=(ci == NTC - 1))
        outrow = spool.tile([1, D], F32, tag="outrow")
        nc.vector.tensor_scalar(outrow[:], po[:], rz[:], None, op0=ALU.mult)
        nc.vector.tensor_add(outrow[:], outrow[:], prerow[:])
        if debug_taps:
            nc.sync.dma_start(dbg["dbg_srow"].ap()[:], f32(srow[:]))
            nc.sync.dma_start(dbg["dbg_spt"].ap()[:], spt[:])
            nc.sync.dma_start(dbg["dbg_wpt"].ap()[:], wpt[:])
            nc.sync.dma_start(dbg["dbg_qcol"].ap()[:], zlast[:])
            nc.sync.dma_start(dbg["dbg_ucol"].ap()[:], ucol[:])
            nc.sync.dma_start(dbg["dbg_vtm"].ap()[:], vtm[:])
            nc.sync.dma_start(dbg["dbg_zsum"].ap()[:], pre[:])
        nc.sync.dma_start(out_d.ap()[None, :], outrow[:])

    nc.compile()
    return nc


def get_program(debug_taps=False):
    key = 'nc_dbg' if debug_taps else 'nc'
    if key not in _CACHE:
        _CACHE[key] = _build_program(debug_taps)
    return _CACHE[key]


def _pack_chunked(w):
    """[d_out, c_in] (512x512) -> [128, 4*512] with [p, cc*512+m] = w[cc*128+p, m].

    Pass w already oriented so that rows are the matmul contraction dim.
    """
    return np.ascontiguousarray(
        w.reshape(NCH, 128, D).transpose(1, 0, 2).reshape(128, NCH * D))


def _pack_conv(w):
    """[C_out, C_in=512, KS] -> [128, (cc, k, m)] with
    [p, (cc*KS+k)*512 + m] = w[m, cc*128+p, KS-1-k].

    Taps are stored reversed: XLA conv (cross-correlation) applies tap j to
    x[t - (KS-1-j)*dil], while the kernel shifts tap k by k*dil.
    """
    wt = w[:, :, ::-1].transpose(1, 2, 0)           # [cin, k, cout]
    wt = wt.reshape(NCH, 128, KS, D).transpose(1, 0, 2, 3)  # [p, cc, k, m]
    return np.ascontiguousarray(wt.reshape(128, NCH * KS * D))


def _bias_col(v):
    return np.ascontiguousarray(v.reshape(NCH, 128).T)


def make_in_maps(x, c1w0, c1b0, c2w0, c2b0, resw, resb, c1w, c1b, c2w, c2b,
                 wq, bq, wk, bk, wv, bv, wp, bp):
    import ml_dtypes
    bf16 = ml_dtypes.bfloat16
    f = lambda a: np.asarray(a, dtype=np.float32)
    x = f(x)

    # block0 conv1 taps packed: part 0-63 tap@t (w[..,2]), part 64-127
    # tap@t-1 (w[..,1]); second D block, part 0-63: tap@t-2 (w[..,0]).
    w0 = f(c1w0)
    w0c1 = np.zeros((128, 2 * D), np.float32)
    w0c1[0:64, 0:D] = w0[:, :, 2].T
    w0c1[64:128, 0:D] = w0[:, :, 1].T
    w0c1[0:64, D:2 * D] = w0[:, :, 0].T
    w0c1 = w0c1.astype(bf16)
    wres_p = np.ascontiguousarray(f(resw)[:, :, 0].T).astype(bf16)

    convs = [f(c2w0)]
    for i in range(N_LAYERS - 1):
        convs.append(f(c1w)[i])
        convs.append(f(c2w)[i])
    wmain = np.stack([_pack_conv(w) for w in convs]).astype(bf16)

    wu = (f(wk).astype(np.float64).T @ f(wq).astype(np.float64)).astype(np.float32)
    wpv = (f(wp).astype(np.float64) @ f(wv).astype(np.float64)).astype(np.float32)
    wu_p = _pack_chunked(wu.T).astype(bf16)         # lhsT tiles for u = Wu z_last
    wpv_p = _pack_chunked(wpv.T).astype(bf16)       # rhs tiles for V' = (Wp Wv) z
    wp_p = _pack_chunked(f(wp).T)                   # lhsT tiles for pre

    bvecs = [f(c1b0), f(c2b0), f(resb)]
    for i in range(N_LAYERS - 1):
        bvecs.append(f(c1b)[i])
        bvecs.append(f(c2b)[i])
    bu = (f(wk).astype(np.float64).T @ f(bq).astype(np.float64)).astype(np.float32)
    bvecs += [bu, f(bk), f(bv), f(bp)]
    bcol = np.concatenate([_bias_col(v) for v in bvecs], axis=1)

    in_maps = []
    for b in range(B):
        xb = x[b].T  # [64, T]
        xcm = np.zeros((128, CT), np.float32)
        xcm[0:64, PAD:] = xb
        xcm[64:128, PAD:] = np.pad(xb, ((0, 0), (1, 0)))[:, :T]   # x(t-1)
        in_maps.append({
            "xcm": np.ascontiguousarray(xcm).astype(bf16),
            "ones": np.ones((1, 128), np.float32),
            "zpad": np.zeros((128, PAD), bf16),
            "w0c1": w0c1,
            "wres": wres_p,
            "wmain": wmain,
            "wu": wu_p,
            "wpv": wpv_p,
            "wp": wp_p,
            "bcol": bcol,
        })
    return in_maps


def _enable_ldw_opt():
    """Let walrus dedupe consecutive same-stationary LDWEIGHTS: our conv
    emits 4 back-to-back matmuls per weight tile, so the reload is pure
    overhead. The flag is hardcoded off in bass_utils; rewrite it on the
    walrus_driver command line for our own compiles."""
    from concourse import bass_utils
    if getattr(bass_utils, '_ldw_patched', False):
        return
    orig = bass_utils.run_command

    def run_command_ldw(cmd, cwd=None, **kw):
        cmd = ['--enable-ldw-opt=true' if c == '--enable-ldw-opt=false' else c
               for c in cmd]
        return orig(cmd, cwd=cwd, **kw)

    bass_utils.run_command = run_command_ldw
    bass_utils._ldw_patched = True


def kernel(**inputs):
    from concourse import bass_utils
    nc = get_program()
    in_maps = make_in_maps(**inputs)
    res = bass_utils.run_bass_kernel_spmd(nc, in_maps, core_ids=list(range(B)))
    out = np.stack([res.results[b]["out"] for b in range(B)], axis=0)
    return out.astype(np.float32)

